# revision 22
# baseline (speedup 1.0000x reference)
"""Trainium2 Bass kernel for MicroNetInt8 (LLM.int8-style quantized linear).

Computes, for x [32768,1,28,28] f32, w_q [1000,784] int8, scb [1000] f32,
bias [1000] f32:
    xf  = x.reshape(B, 784)
    ax  = max(|xf|, axis=1)                      (clamped; randn never hits 0)
    x_q = round(xf * 127/ax)                     (int8 range, kept in bf16)
    acc = x_q @ w_q.T                            (exact: ints < 2^24 in fp32)
    y   = relu(acc * (ax/127) * (scb/127) + bias)

Sharding: pure data parallel, batch split 8 ways (4096 rows/core); the tiny
weight is replicated. No collectives.

Device-side trick: the bias is folded into the matmul as an extra
contraction row (w row 784 = bias/s_o, x col 784 = 127/ax), so the epilogue
is y = s_o * relu(acc * ax/127) — one ACT op (relu, per-partition scale)
plus one DVE multiply (per-column scale).
"""

import sys
import types

sys.path.insert(0, "/opt/trn_rl_repo")

import numpy as np
import ml_dtypes

N_CORES = 8
B_FULL = 32768
IN = 784
OUT = 1000
B_SHARD = B_FULL // N_CORES          # 4096
TILE_B = 128
N_TILES = B_SHARD // TILE_B          # 32
KAUG = IN + 1                        # 785: augmented contraction (bias row)
KCH = (KAUG + 127) // 128            # 7 chunks of the contraction dim
NSPLIT = OUT // 2                    # 500 <= 512 fp32 per PSUM bank
Q = np.float32(127.0)
MAGIC = 1536.0                       # fp16 magic: ulp=1 in [1024,2048)

_CACHE = {}


def _ensure_axon_hooks():
    """Install the NTFF profile hook if the image's antenv lacks it."""
    if "antenv.axon_hooks" in sys.modules:
        return
    try:
        import antenv
    except ImportError:
        return
    m = types.ModuleType("antenv.axon_hooks")
    _hook = [None]
    m.set_axon_ntff_profile_hook = lambda h: _hook.__setitem__(0, h)
    m.get_axon_ntff_profile_hook = lambda: _hook[0]
    sys.modules["antenv.axon_hooks"] = m
    antenv.axon_hooks = m
    try:
        from trn_agent_boot.trn_boot import _ntff_profile_via_ctypes

        h = _ntff_profile_via_ctypes("/opt/axon/libaxon_pjrt.so")
        if h is not None:
            m.set_axon_ntff_profile_hook(h)
    except Exception:
        pass


def _build():
    from contextlib import ExitStack

    import concourse.bacc as bacc
    import concourse.tile as tile
    from concourse.tile import add_dep_helper
    from concourse import mybir

    f32 = mybir.dt.float32
    f16 = mybir.dt.float16
    bf16 = mybir.dt.bfloat16

    nc = bacc.Bacc("TRN2", target_bir_lowering=False, debug=False)
    x_ap = nc.dram_tensor("x", [B_SHARD, IN], f32, kind="ExternalInput").ap()
    w_ap = nc.dram_tensor("w", [128, KCH, OUT], bf16, kind="ExternalInput").ap()
    so_ap = nc.dram_tensor("so", [OUT], f32, kind="ExternalInput").ap()
    id_ap = nc.dram_tensor("ident", [128, 128], f16, kind="ExternalInput").ap()
    out_ap = nc.dram_tensor("out", [B_SHARD, OUT], f32, kind="ExternalOutput").ap()

    with tile.TileContext(nc) as tc, ExitStack() as ctx:
        consts = ctx.enter_context(tc.tile_pool(name="consts", bufs=1))
        w_sb = consts.tile([128, KCH, OUT], bf16)
        so_sb = consts.tile([128, OUT], f32)
        id_sb = consts.tile([128, 128], f16)

        xpool = ctx.enter_context(tc.tile_pool(name="xin", bufs=4))
        # x0 and x1 go out first on the sync ring: the first two tiles'
        # input stages gate the whole pipeline startup
        x0 = xpool.tile([TILE_B, IN], f32, tag="xt")
        nc.sync.dma_start(x0[:], x_ap[0:TILE_B, :])
        nc.scalar.dma_start(id_sb[:], id_ap[:])
        # w chunk 0 first so the first product matmuls don't wait for the
        # whole 1.75MB weight transfer
        nc.scalar.dma_start(w_sb[:, 0:1, :], w_ap[:, 0:1, :])
        nc.scalar.dma_start(w_sb[:, 1:KCH, :], w_ap[:, 1:KCH, :])
        qpool = ctx.enter_context(tc.tile_pool(name="quant", bufs=3))
        tpool = ctx.enter_context(tc.tile_pool(name="tiny", bufs=4))
        opool = ctx.enter_context(tc.tile_pool(name="outp", bufs=3))
        pst_pool = ctx.enter_context(tc.tile_pool(name="psT", bufs=2, space="PSUM"))
        psm_pool = ctx.enter_context(tc.tile_pool(name="psM", bufs=3, space="PSUM"))

        # ACT handles the first ACT_SPLIT columns of the -MAGIC copy; DVE the rest
        ACT_SPLIT = 320

        def stage_in(t, xt=None):
            """DMA + absmax + scales + quantize (pre-round, +MAGIC)."""
            row = t * TILE_B
            if xt is None:
                xt = xpool.tile([TILE_B, IN], f32, tag="xt")
                nc.sync.dma_start(xt[:], x_ap[row : row + TILE_B, :])
            ax = tpool.tile([TILE_B, 1], f32, tag="ax")
            red_i = nc.vector.tensor_reduce(
                ax[:], xt[:], axis=mybir.AxisListType.X,
                op=mybir.AluOpType.max, apply_absolute_value=True,
            )
            u = tpool.tile([TILE_B, 1], f32, tag="u")
            nc.vector.tensor_scalar_mul(u[:], ax[:], 1.0 / 127.0)
            rcp = tpool.tile([TILE_B, 1], f32, tag="rcp")
            nc.vector.reciprocal(rcp[:], u[:])
            q1 = qpool.tile([TILE_B, KAUG], f16, tag="q1")
            quant_i = nc.scalar.activation(
                q1[:, 0:IN], xt[:], mybir.ActivationFunctionType.Copy,
                bias=MAGIC, scale=rcp[:],
            )
            nc.vector.tensor_scalar_add(q1[:, IN : IN + 1], rcp[:], MAGIC)
            return dict(u=u, q1=q1, quant_i=quant_i, red_i=red_i)

        def stage_transpose(s):
            psT = pst_pool.tile([TILE_B, KCH * 128], f16, tag="psT")
            for c in range(KCH):
                k0 = c * 128
                kc = min(128, KAUG - k0)
                nc.tensor.transpose(
                    psT[0:kc, k0 : k0 + TILE_B],
                    s["q1"][:, k0 : k0 + kc],
                    id_sb[:],
                )
            s["psT"] = psT

        def stage_subm(s):
            """-MAGIC + bf16 convert (PSUM->SBUF), split ACT/DVE."""
            psT = s["psT"]
            xqT = qpool.tile([TILE_B, KCH * 128], bf16, tag="xqT")
            s["subm_act"] = nc.scalar.activation(
                xqT[:, 0:ACT_SPLIT], psT[:, 0:ACT_SPLIT],
                mybir.ActivationFunctionType.Copy, bias=-MAGIC, scale=1.0,
            )
            nc.vector.tensor_scalar_sub(
                xqT[:, ACT_SPLIT : 6 * 128], psT[:, ACT_SPLIT : 6 * 128], MAGIC
            )
            s["subm_dve"] = nc.vector.tensor_scalar_sub(
                xqT[0 : KAUG - 768, 6 * 128 :], psT[0 : KAUG - 768, 6 * 128 :], MAGIC
            )
            s["xqT"] = xqT

        def stage_mm(s):
            xqT = s["xqT"]
            psA = psm_pool.tile([TILE_B, NSPLIT], f32, tag="psA")
            psB = psm_pool.tile([TILE_B, NSPLIT], f32, tag="psB")
            for c in range(KCH):
                k0 = c * 128
                kc = min(128, KAUG - k0)
                lhsT = xqT[0:kc, k0 : k0 + TILE_B]
                nc.tensor.matmul(
                    psA[:], lhsT, w_sb[0:kc, c : c + 1, 0:NSPLIT],
                    start=(c == 0), stop=(c == KCH - 1),
                )
                nc.tensor.matmul(
                    psB[:], lhsT, w_sb[0:kc, c : c + 1, NSPLIT:OUT],
                    start=(c == 0), stop=(c == KCH - 1),
                )
            s["psA"], s["psB"] = psA, psB

        def stage_out(t, s, nxt):
            """y = relu(acc) * (s_o * ax/127); bias folded into acc."""
            rs = opool.tile([TILE_B, OUT], f32, tag="rs")
            rs_i = nc.scalar.activation(
                rs[:], so_sb[:], mybir.ActivationFunctionType.Copy,
                bias=0.0, scale=s["u"][:],
            )
            # rs has ~3us of slack (only needed by the STT after the
            # matmuls); keep it off the subM->quant critical chain
            add_dep_helper(rs_i.ins, s["subm_act"].ins, sync=False,
                           reason="rs after subM on ACT")
            if nxt is not None:
                add_dep_helper(rs_i.ins, nxt["quant_i"].ins, sync=False,
                               reason="rs after next quant on ACT")
            y = opool.tile([TILE_B, OUT], f32, tag="y")
            nc.vector.scalar_tensor_tensor(
                y[:, 0:NSPLIT], s["psA"][:], 0.0, rs[:, 0:NSPLIT],
                op0=mybir.AluOpType.max, op1=mybir.AluOpType.mult,
            )
            nc.vector.scalar_tensor_tensor(
                y[:, NSPLIT:OUT], s["psB"][:], 0.0, rs[:, NSPLIT:OUT],
                op0=mybir.AluOpType.max, op1=mybir.AluOpType.mult,
            )
            row = t * TILE_B
            nc.sync.dma_start(out_ap[row : row + TILE_B, :], y[:])

        # software pipeline: transposes of tile t+1 are emitted (and run on
        # the PE) before the matmuls of tile t; subM of tile t is emitted
        # before the input stage of tile t+1 so the in-order ACT/DVE queues
        # keep the PE fed.
        cur = stage_in(0, xt=x0)
        stage_transpose(cur)
        for t in range(N_TILES):
            stage_subm(cur)
            if t + 1 < N_TILES:
                nxt = stage_in(t + 1)
                add_dep_helper(nxt["quant_i"].ins, cur["subm_act"].ins,
                               sync=False, reason="act pipeline order")
                add_dep_helper(nxt["red_i"].ins, cur["subm_dve"].ins,
                               sync=False, reason="dve pipeline order")
                if t == 0:
                    # so lands after x1 on the sync ring; it is only needed
                    # by rs0 (which runs after quant1 anyway)
                    nc.sync.dma_start(
                        so_sb[:], so_ap[None].broadcast_to([128, OUT])
                    )
                stage_transpose(nxt)
            else:
                nxt = None
            stage_mm(cur)
            stage_out(t, cur, nxt)
            if nxt is not None:
                cur = nxt

    nc.compile()
    return nc


def _pack_inputs(x, w_q, scb, bias):
    xf = np.ascontiguousarray(x.reshape(B_FULL, IN).astype(np.float32, copy=False))
    so = (scb.astype(np.float32) / Q).astype(np.float32)
    w_aug = np.zeros((KCH * 128, OUT), np.float32)
    w_aug[:IN, :] = w_q.T.astype(np.float32)
    w_aug[IN, :] = bias.astype(np.float32) / so
    w_pack = np.ascontiguousarray(
        w_aug.reshape(KCH, 128, OUT).transpose(1, 0, 2)
    ).astype(ml_dtypes.bfloat16)
    ident = np.eye(128, dtype=np.float16)
    in_maps = []
    for c in range(N_CORES):
        in_maps.append(
            {
                "x": xf[c * B_SHARD : (c + 1) * B_SHARD],
                "w": w_pack,
                "so": so,
                "ident": ident,
            }
        )
    return in_maps


def _get_compiled():
    if "nc" not in _CACHE:
        _ensure_axon_hooks()
        _CACHE["nc"] = _build()
    return _CACHE["nc"]


def run_sharded(x, w_q, scb, bias, trace=False, **kw):
    """Compile (cached), run on 8 NeuronCores, return BassKernelResults."""
    from concourse import bass_utils

    bass_utils.upload_artifacts = lambda tmpdir: "local://" + tmpdir
    nc = _get_compiled()
    in_maps = _pack_inputs(x, w_q, scb, bias)
    return bass_utils.run_bass_kernel_spmd(
        nc, in_maps, list(range(N_CORES)), trace=trace, **kw
    )


def kernel(x, w_q, scb, bias):
    res = run_sharded(x, w_q, scb, bias, trace=False)
    return np.concatenate(
        [res.results[c]["out"] for c in range(N_CORES)], axis=0
    )
